# revision 11
# baseline (speedup 1.0000x reference)
"""Trainium2 Bass kernel for nn_BlockAttentionResidual.

Reference semantics (per batch sample b):
  V[n]   = concat(blocks[:, b], x[b][None])          # (9, C, H, W)
  mean_n, var_n = moments over (C, H, W) of V[n]     # GroupNorm(1, C)
  K[n]   = (V[n] - mean_n) * rsqrt(var_n + eps) * gamma + beta
  l[n]   = sum_c w[c] * K[n, c]                      # (H, W) logits
  attn   = softmax_n(l)
  h[b]   = sum_n attn[n] * V[n]

Sharding: 8 cores = (B=4) x (H halves).  Core k handles b = k//2 and
rows [32*(k%2), 32*(k%2)+32).  GroupNorm statistics reduce over the full
sample, so the two cores sharing a b exchange per-slice partial sums
(s1, s2) with a tiny AllReduce; everything else is core-local.

Per-core dataflow (streamed over n; online softmax without max-shift --
logits are O(0.3) so exp() is safe):
  DMA   : V[n] slice (128c x 2048s) f32 tiles, c on partitions
  PE    : proj[n, s] = sum_c weff_c V[n, c, s]   (float32r matmul)
  DVE   : cast V -> bf16 with accum_out = per-partition sum  (s1 partial)
  ACT   : Square with accum_out = per-partition sum of squares (s2 partial)
  CC    : AllReduce(add) of the (128, 4) partial columns between the b-pair
  small : mean/var/invstd -> a_n, bias_n; e[n] = Exp(a_n * proj + bias_n)
  GPSIMD: broadcast e[n] row across 128 partitions (bf16)
  DVE   : R += V_bf16[n] * e_bcast[n]
  tail  : Z = sum_n e[n] (ones matmul), Zinv = exp(-ln Z), h = R * Zinv
"""

import numpy as np

import concourse.bacc as bacc
import concourse.bass as bass
import concourse.mybir as mybir
import concourse.tile as tile
from concourse import bass_utils
from concourse._compat import with_exitstack

F32 = mybir.dt.float32
F32R = mybir.dt.float32r
BF16 = mybir.dt.bfloat16
AF = mybir.ActivationFunctionType
ALU = mybir.AluOpType
AX = mybir.AxisListType

N1 = 9           # 8 blocks + x
C = 256
CC = 2           # channel chunks of 128
P = 128
S = 2048         # spatial elements per core (32 rows x 64 cols)
NSJ = 4          # 512-wide spatial chunks for matmul / psum banks
SJ = 512
MTOT = float(C * 64 * 64)   # GroupNorm reduction count (full sample)
EPS = 1e-5
REPLICA_GROUPS = [[0, 1], [2, 3], [4, 5], [6, 7]]


@with_exitstack
def _emit(ctx, tc, v, w, gnw, gnb, hout, rep=0):
    nc = tc.nc

    def pool(name, bufs, **kw):
        return ctx.enter_context(tc.tile_pool(name=f"{name}{rep}", bufs=bufs, **kw))

    const = pool("const", 1)
    vpool = pool("vraw", 4)
    vbpool = pool("vbf", 8)
    scr = pool("scr", 2)
    ebp = pool("ebp", 3)
    prodp = pool("prod", 3)
    rp = pool("racc", 1)
    sp = pool("small", 3)
    hp = pool("hout", 2)
    psp = pool("ps", 2, space="PSUM")
    dp = pool("dram", 4, space="DRAM")

    # ---- setup: weff = w * gamma (128, 2 chunks); sw = sum(weff); cbias = sum(w*beta)
    wt = const.tile([P, 2], F32, tag="wt")
    nc.sync.dma_start(wt[:], w.rearrange("(a b) -> b a", a=2))
    gwt = const.tile([P, 2], F32, tag="gwt")
    nc.sync.dma_start(gwt[:], gnw.rearrange("(a b) -> b a", a=2))
    gbt = const.tile([P, 2], F32, tag="gbt")
    nc.sync.dma_start(gbt[:], gnb.rearrange("(a b) -> b a", a=2))

    wcomb = const.tile([P, 4], F32, tag="wcomb")   # [weff0, weff1, wbeta0, wbeta1]
    nc.vector.tensor_tensor(wcomb[:, 0:2], wt[:], gwt[:], ALU.mult)
    nc.vector.tensor_tensor(wcomb[:, 2:4], wt[:], gbt[:], ALU.mult)
    weff_bf = const.tile([P, 2], BF16, tag="weff_bf")
    nc.vector.tensor_copy(weff_bf[:], wcomb[:, 0:2])

    # cross-partition totals via a DRAM roundtrip (transposed readback)
    wscr = dp.tile([P, 4], F32, tag="wscr")
    nc.sync.dma_start(wscr[:], wcomb[:])
    wrb = sp.tile([1, 512], F32, tag="rb")
    nc.sync.dma_start(wrb[:], wscr[:].rearrange("a (j c) -> j c a", j=2))
    swcb = const.tile([1, 2], F32, tag="swcb")     # [sum(weff), sum(w*beta)]
    nc.vector.tensor_reduce(swcb[:], wrb[:].rearrange("a (j r) -> a j r", j=2),
                            AX.X, ALU.add)
    sw_ap = swcb[:, 0:1]
    cb_ap = swcb[:, 1:2]

    ones_bf = const.tile([1, 1], BF16, tag="ones_bf")
    nc.vector.memset(ones_bf[:], 1.0)

    # R accumulators (ping-pong per channel chunk)
    r_tiles = [[rp.tile([P, S], BF16, tag=f"r{cc}{j}", name=f"r{cc}{j}")
                for j in range(2)] for cc in range(CC)]
    for cc in range(CC):
        nc.gpsimd.memset(r_tiles[cc][0][:], 0.0)

    # packed e rows: partition n holds e[n] (filled via tiny DMAs)
    e_all = sp.tile([N1, S], BF16, tag="e_all", bufs=1)

    for n in range(N1):
        vts = []
        for cc in range(CC):
            vt = vpool.tile([P, S], F32, tag="v")
            nc.sync.dma_start(vt[:], v[n, cc * P:(cc + 1) * P, :])
            vts.append(vt)

        # DVE cast->bf16 (+ s1 partials), ACT square (+ s2 partials)
        statp = sp.tile([P, 4], F32, tag="statp")
        vbts = []
        for cc in range(CC):
            vbt = vbpool.tile([P, S], BF16, tag="vb")
            nc.vector.tensor_scalar(vbt[:], vts[cc][:], 1.0, None, ALU.mult,
                                    ALU.add, accum_out=statp[:, cc:cc + 1])
            vbts.append(vbt)
            sq = scr.tile([P, S], BF16, tag="sq")
            nc.scalar.activation(sq[:], vts[cc][:], AF.Square,
                                 accum_out=statp[:, 2 + cc:3 + cc])

        # PE: proj[n, s] accumulated over both channel chunks (bf16 inputs)
        pr = psp.tile([1, S], F32, tag="pr")
        for sj in range(NSJ):
            sl = slice(sj * SJ, (sj + 1) * SJ)
            for cc in range(CC):
                nc.tensor.matmul(
                    pr[:, sl],
                    weff_bf[:, cc:cc + 1],
                    vbts[cc][:, sl],
                    start=(cc == 0), stop=(cc == 1),
                )

        # pairwise AllReduce of the partial-sum columns
        bin_d = dp.tile([P, 4], F32, tag="bin")
        bout_d = dp.tile([P, 4], F32, tag="bout")
        nc.sync.dma_start(bin_d[:], statp[:])
        nc.gpsimd.collective_compute(
            "AllReduce", ALU.add, replica_groups=REPLICA_GROUPS,
            ins=[bin_d[:].opt()], outs=[bout_d[:].opt()],
        )
        rb = sp.tile([1, 512], F32, tag="rb")
        nc.sync.dma_start(rb[:], bout_d[:].rearrange("a (j c) -> j c a", j=2))
        st2 = sp.tile([1, 2], F32, tag="st2")   # [s1, s2] over the full sample
        nc.vector.tensor_reduce(st2[:], rb[:].rearrange("a (j r) -> a j r", j=2),
                                AX.X, ALU.add)

        # scalar chain: a_n = rsqrt(var+eps) = exp(-0.5*ln(var+eps))
        mean = sp.tile([1, 1], F32, tag="mean")
        nc.vector.tensor_scalar_mul(mean[:], st2[:, 0:1], 1.0 / MTOT)
        ex2 = sp.tile([1, 1], F32, tag="ex2")
        nc.vector.tensor_scalar_mul(ex2[:], st2[:, 1:2], 1.0 / MTOT)
        m2 = sp.tile([1, 1], F32, tag="m2")
        nc.vector.tensor_tensor(m2[:], mean[:], mean[:], ALU.mult)
        veps = sp.tile([1, 1], F32, tag="veps")
        nc.vector.tensor_tensor(veps[:], ex2[:], m2[:], ALU.subtract)
        nc.vector.tensor_scalar_add(veps[:], veps[:], EPS)
        lnt = sp.tile([1, 1], F32, tag="lnt")
        nc.scalar.activation(lnt[:], veps[:], AF.Ln)
        a_n = sp.tile([1, 1], F32, tag="a_n")
        nc.scalar.activation(a_n[:], lnt[:], AF.Exp, scale=-0.5)
        # bias_n = cbias - a_n * sw * mean
        asw = sp.tile([1, 1], F32, tag="asw")
        nc.vector.tensor_tensor(asw[:], a_n[:], sw_ap, ALU.mult)
        nc.vector.tensor_tensor(asw[:], asw[:], mean[:], ALU.mult)
        bias_n = sp.tile([1, 1], F32, tag="bias_n")
        nc.vector.tensor_tensor(bias_n[:], cb_ap, asw[:], ALU.subtract)

        # e[n] = exp(a_n * proj + bias_n)  (bf16 row)
        e_row = sp.tile([1, S], BF16, tag="e_row")
        nc.scalar.activation(e_row[:], pr[:], AF.Exp, bias=bias_n[:], scale=a_n[:])
        nc.sync.dma_start(e_all[n:n + 1, :], e_row[:])

        # broadcast e across partitions; R += V_bf16 * e
        eb = ebp.tile([P, S], BF16, tag="eb")
        nc.gpsimd.partition_broadcast(eb[:], e_row[:], channels=P)
        for cc in range(CC):
            pt = prodp.tile([P, S], BF16, tag="pt")
            nc.vector.tensor_tensor(pt[:], vbts[cc][:], eb[:], ALU.mult)
            nc.vector.tensor_tensor(r_tiles[cc][(n + 1) % 2][:],
                                    r_tiles[cc][n % 2][:], pt[:], ALU.add)

    # ---- tail: Z, Zinv, h = R * Zinv
    zps = psp.tile([1, S], F32, tag="pr")
    ones9 = const.tile([N1, 1], BF16, tag="ones9")
    nc.vector.memset(ones9[:], 1.0)
    for sj in range(NSJ):
        sl = slice(sj * SJ, (sj + 1) * SJ)
        nc.tensor.matmul(zps[:, sl], ones9[:], e_all[:, sl], start=True, stop=True)
    lnz = hp.tile([1, S], F32, tag="ho")
    nc.scalar.activation(lnz[:], zps[:], AF.Ln)
    ziv = hp.tile([1, S], F32, tag="ho")
    nc.scalar.activation(ziv[:], lnz[:], AF.Exp, scale=-1.0)
    zib = const.tile([P, S], F32, tag="zib")
    nc.gpsimd.partition_broadcast(zib[:], ziv[:], channels=P)

    for cc in range(CC):
        ho = hp.tile([P, S], F32, tag="ho")
        nc.vector.tensor_tensor(ho[:], r_tiles[cc][N1 % 2][:], zib[:], ALU.mult)
        nc.sync.dma_start(hout[cc * P:(cc + 1) * P, :], ho[:])


_CACHE = {}


def _build(reps=1):
    key = ("nc", reps)
    if key in _CACHE:
        return _CACHE[key]
    nc = bacc.Bacc("TRN2", target_bir_lowering=False, debug=False, num_devices=8)
    v = nc.dram_tensor("v", [N1, C, S], F32, kind="ExternalInput").ap()
    w = nc.dram_tensor("w", [C], F32, kind="ExternalInput").ap()
    gnw = nc.dram_tensor("gnw", [C], F32, kind="ExternalInput").ap()
    gnb = nc.dram_tensor("gnb", [C], F32, kind="ExternalInput").ap()
    hout = nc.dram_tensor("h", [C, S], F32, kind="ExternalOutput").ap()
    with tile.TileContext(nc) as tc:
        for r in range(reps):
            _emit(tc, v, w, gnw, gnb, hout, rep=r)
    nc.compile()
    _CACHE[key] = nc
    return nc


def make_in_maps(blocks, x, w, gn_weight, gn_bias):
    blocks = np.asarray(blocks, dtype=np.float32)
    x = np.asarray(x, dtype=np.float32)
    w = np.asarray(w, dtype=np.float32)
    gn_weight = np.asarray(gn_weight, dtype=np.float32)
    gn_bias = np.asarray(gn_bias, dtype=np.float32)
    in_maps = []
    for k in range(8):
        b, hh = k // 2, k % 2
        hs = 32 * hh
        arr = np.empty((N1, C, S), np.float32)
        arr[:8] = blocks[:, b, :, hs:hs + 32, :].reshape(8, C, S)
        arr[8] = x[b, :, hs:hs + 32, :].reshape(C, S)
        in_maps.append({"v": arr, "w": w, "gnw": gn_weight, "gnb": gn_bias})
    return in_maps


def assemble(results):
    h = np.empty((4, C, 64, 64), np.float32)
    for k in range(8):
        b, hh = k // 2, k % 2
        h[b, :, 32 * hh:32 * hh + 32, :] = results[k]["h"].reshape(C, 32, 64)
    return h


def kernel(blocks, x, w, gn_weight, gn_bias):
    nc = _build()
    in_maps = make_in_maps(blocks, x, w, gn_weight, gn_bias)
    res = bass_utils.run_bass_kernel_spmd(nc, in_maps, core_ids=list(range(8)))
    return assemble(res.results)


# revision 17
# speedup vs baseline: 1.2312x; 1.2312x over previous
"""Trainium2 Bass kernel for nn_BlockAttentionResidual.

Reference semantics (per batch sample b):
  V[n]   = concat(blocks[:, b], x[b][None])          # (9, C, H, W)
  mean_n, var_n = moments over (C, H, W) of V[n]     # GroupNorm(1, C)
  K[n]   = (V[n] - mean_n) * rsqrt(var_n + eps) * gamma + beta
  l[n]   = sum_c w[c] * K[n, c]                      # (H, W) logits
  attn   = softmax_n(l)
  h[b]   = sum_n attn[n] * V[n]

Sharding: 8 cores = (B=4) x (H halves).  Core k handles b = k//2 and
rows [32*(k%2), 32*(k%2)+32).  GroupNorm statistics reduce over the full
sample, so the two cores sharing a b exchange per-slice partial sums
(s1, s2) with a tiny AllReduce; everything else is core-local.

Per-core dataflow (streamed over n; online softmax without max-shift --
logits are O(0.3) so exp() is safe):
  DMA   : V[n] slice as one (128, 2, 2048) f32 tile (c-chunk on middle dim)
  DVE   : cast V -> bf16 with accum_out = per-partition sum  (s1 partial)
  ACT   : Square with accum_out = per-partition sum of squares (s2 partial)
  PE    : proj[n, s] = sum_c weff_c V[n, c, s]   (bf16 matmul)
  CC    : AllReduce(add) of the (128, 2) partial columns between the b-pair
  small : mean/var -> a_n = exp(-0.5 ln(var+eps)), bias_n; e = Exp(a*proj+b)
  GPSIMD: broadcast e[n] row across 128 partitions (bf16)
  DVE   : R += V_bf16[n] * e_bcast[n]
  tail  : Z = sum_n e[n] (ones matmul), Zinv = exp(-ln Z), h = R * Zinv
"""

import numpy as np

import concourse.bacc as bacc
import concourse.bass as bass
import concourse.hw_specs as hw_specs
import concourse.mybir as mybir
import concourse.tile as tile
from concourse import bass_utils
from concourse._compat import with_exitstack

F32 = mybir.dt.float32
BF16 = mybir.dt.bfloat16
AF = mybir.ActivationFunctionType
ALU = mybir.AluOpType
AX = mybir.AxisListType

N1 = 9           # 8 blocks + x
C = 256
CC = 2           # channel chunks of 128
P = 128
S = 2048         # spatial elements per core (32 rows x 64 cols)
NSJ = 4          # 512-wide spatial chunks for matmul / psum banks
SJ = 512
MTOT = float(C * 64 * 64)   # GroupNorm reduction count (full sample)
EPS = 1e-5
REPLICA_GROUPS = [[0, 1], [2, 3], [4, 5], [6, 7]]

# Pin all activations (Square/Ln/Exp) to the one table set that contains
# them all; the default greedy per-instruction chooser alternates between
# exp_and_others and natural_log_exp_and_others, paying a ~2.7us table
# load per switch.  Claiming the other sets lack these functions is
# conservative and only affects load placement.
_ORIG_TABLES = hw_specs.get_activation_tables


def _pinned_tables(arch):
    t = _ORIG_TABLES(arch)
    keep = "natural_log_exp_and_others"
    if keep not in t:
        return t
    kf = t[keep]
    return {k: (v if k == keep else (v - kf)) for k, v in t.items()}


bacc.get_activation_tables = _pinned_tables


@with_exitstack
def _emit(ctx, tc, v, w, gnw, gnb, hout, rep=0, no_cc=False):
    nc = tc.nc

    def pool(name, bufs, **kw):
        return ctx.enter_context(tc.tile_pool(name=f"{name}{rep}", bufs=bufs, **kw))

    const = pool("const", 1)
    vpool = pool("vraw", 2)
    vbpool = pool("vbf", 4)
    scr = pool("scr", 2)
    ebp = pool("ebp", 3)
    prodp = pool("prod", 2)
    rp = pool("racc", 1)
    sp = pool("small", 3)
    hp = pool("hout", 2)
    psp = pool("ps", 2, space="PSUM")
    dp = pool("dram", 4, space="DRAM")

    # ---- setup: weff = w * gamma (128, 2 chunks); sw = sum(weff); cbias = sum(w*beta)
    wt = const.tile([P, 2], F32, tag="wt")
    nc.sync.dma_start(wt[:], w.rearrange("(a b) -> b a", a=2))
    gwt = const.tile([P, 2], F32, tag="gwt")
    nc.sync.dma_start(gwt[:], gnw.rearrange("(a b) -> b a", a=2))
    gbt = const.tile([P, 2], F32, tag="gbt")
    nc.sync.dma_start(gbt[:], gnb.rearrange("(a b) -> b a", a=2))

    wcomb = const.tile([P, 4], F32, tag="wcomb")   # [weff0, weff1, wbeta0, wbeta1]
    nc.vector.tensor_tensor(wcomb[:, 0:2], wt[:], gwt[:], ALU.mult)
    nc.vector.tensor_tensor(wcomb[:, 2:4], wt[:], gbt[:], ALU.mult)
    weff_bf = const.tile([P, 2], BF16, tag="weff_bf")
    nc.vector.tensor_copy(weff_bf[:], wcomb[:, 0:2])

    # cross-partition totals via a DRAM roundtrip (transposed readback)
    wscr = dp.tile([P, 4], F32, tag="wscr")
    nc.sync.dma_start(wscr[:], wcomb[:])
    wrb = sp.tile([1, 512], F32, tag="wrb", bufs=1)
    nc.sync.dma_start(wrb[:], wscr[:].rearrange("a (j c) -> j c a", j=2))
    swcb = const.tile([1, 2], F32, tag="swcb")     # [sum(weff), sum(w*beta)]
    nc.vector.tensor_reduce(swcb[:], wrb[:].rearrange("a (j r) -> a j r", j=2),
                            AX.X, ALU.add)
    sw_ap = swcb[:, 0:1]
    cb_ap = swcb[:, 1:2]

    ones_bf = const.tile([N1, 1], BF16, tag="ones_bf")
    nc.vector.memset(ones_bf[:], 1.0)

    # R accumulators (ping-pong)
    r_tiles = [rp.tile([P, CC, S], BF16, tag=f"r{j}", name=f"r{j}") for j in range(2)]
    nc.gpsimd.memset(r_tiles[0][:], 0.0)

    # packed e rows: partition n holds e[n] (filled via tiny DMAs)
    e_all = sp.tile([N1, S], BF16, tag="e_all", bufs=1)

    def emit_front(n):
        """load slice n, local stats partials, proj matmul, start exchange"""
        vt = vpool.tile([P, CC, S], F32, tag="v", name="vt")
        nc.sync.dma_start(vt[:], v[n].rearrange("(a p) s -> p a s", a=2))

        # DVE cast->bf16 (+ s1 partial), ACT square (+ s2 partial)
        s1p = sp.tile([P, 1], F32, tag="s1p", name="s1p")
        s2p = sp.tile([P, 1], F32, tag="s2p", name="s2p")
        vbt = vbpool.tile([P, CC, S], BF16, tag="vb", name="vbt")
        nc.vector.tensor_scalar(vbt[:], vt[:], 1.0, None, ALU.mult,
                                ALU.add, accum_out=s1p[:])
        sq = scr.tile([P, CC, S], BF16, tag="sq", name="sq")
        nc.scalar.activation(sq[:], vt[:], AF.Square, accum_out=s2p[:])

        # PE: proj[n, s] accumulated over both channel chunks (bf16)
        pr = psp.tile([1, S], F32, tag="pr", name="pr")
        for sj in range(NSJ):
            sl = slice(sj * SJ, (sj + 1) * SJ)
            for cc in range(CC):
                nc.tensor.matmul(
                    pr[:, sl],
                    weff_bf[:, cc:cc + 1],
                    vbt[:, cc, sl],
                    start=(cc == 0), stop=(cc == 1),
                )

        # pairwise AllReduce of the partial-sum columns
        bin_d = dp.tile([P, 2], F32, tag="bin", name="bin_d")
        bout_d = dp.tile([P, 2], F32, tag="bout", name="bout_d")
        nc.sync.dma_start(bin_d[:, 0:1], s1p[:])
        nc.sync.dma_start(bin_d[:, 1:2], s2p[:])
        if no_cc:
            nc.sync.dma_start(bout_d[:], bin_d[:])
        else:
            nc.gpsimd.collective_compute(
                "AllReduce", ALU.add, replica_groups=REPLICA_GROUPS,
                ins=[bin_d[:].opt()], outs=[bout_d[:].opt()],
            )
        return vbt, pr, bout_d

    def emit_back(n, state):
        """finish slice n: stats -> a_n -> e[n] -> R += V*e"""
        vbt, pr, bout_d = state
        rb = sp.tile([1, 256], F32, tag="rb", name="rb")
        nc.sync.dma_start(rb[:], bout_d[:].rearrange("a b -> b a").unsqueeze(0))
        st2 = sp.tile([1, 2], F32, tag="st2", name="st2")  # [s1, s2] full sample
        nc.vector.tensor_reduce(st2[:], rb[:].rearrange("a (j r) -> a j r", j=2),
                                AX.X, ALU.add)

        # scalar chain:
        #   mean = s1/M ; veps = E[x^2]+eps - mean^2
        #   a_n = exp(-0.5*ln(veps)) ; bias_n = cbias + a_n*sw*(-mean)
        mean = sp.tile([1, 1], F32, tag="mean", name="mean")
        nc.vector.tensor_scalar_mul(mean[:], st2[:, 0:1], 1.0 / MTOT)
        nmean = sp.tile([1, 1], F32, tag="nmean", name="nmean")
        nc.vector.tensor_scalar_mul(nmean[:], st2[:, 0:1], -1.0 / MTOT)
        ex2eps = sp.tile([1, 1], F32, tag="ex2eps", name="ex2eps")
        nc.vector.tensor_scalar(ex2eps[:], st2[:, 1:2], 1.0 / MTOT, EPS,
                                ALU.mult, ALU.add)
        veps = sp.tile([1, 1], F32, tag="veps", name="veps")
        nc.vector.scalar_tensor_tensor(veps[:], mean[:], nmean[:, 0:1], ex2eps[:],
                                       ALU.mult, ALU.add)
        lnt = sp.tile([1, 1], F32, tag="lnt", name="lnt")
        nc.scalar.activation(lnt[:], veps[:], AF.Ln)
        a_n = sp.tile([1, 1], F32, tag="a_n", name="a_n")
        nc.scalar.activation(a_n[:], lnt[:], AF.Exp, scale=-0.5)
        asnm = sp.tile([1, 1], F32, tag="asnm", name="asnm")
        nc.vector.scalar_tensor_tensor(asnm[:], a_n[:], sw_ap, nmean[:],
                                       ALU.mult, ALU.mult)
        bias_n = sp.tile([1, 1], F32, tag="bias_n", name="bias_n")
        nc.vector.tensor_tensor(bias_n[:], cb_ap, asnm[:], ALU.add)

        # e[n] = exp(a_n * proj + bias_n)  (bf16 row)
        e_row = sp.tile([1, S], BF16, tag="e_row", name="e_row")
        nc.scalar.activation(e_row[:], pr[:], AF.Exp, bias=bias_n[:], scale=a_n[:])
        nc.sync.dma_start(e_all[n:n + 1, :], e_row[:])

        # broadcast e across partitions; R += V_bf16 * e
        eb = ebp.tile([P, S], BF16, tag="eb", name="eb")
        nc.gpsimd.partition_broadcast(eb[:], e_row[:], channels=P)
        ebx = eb[:].unsqueeze(1).to_broadcast((P, CC, S))
        pt = prodp.tile([P, CC, S], BF16, tag="pt", name="pt")
        nc.vector.tensor_tensor(pt[:], vbt[:], ebx, ALU.mult)
        nc.vector.tensor_tensor(r_tiles[(n + 1) % 2][:], r_tiles[n % 2][:],
                                pt[:], ALU.add)

    LAG = 2
    states = {}
    for i in range(N1 + LAG):
        if i < N1:
            states[i] = emit_front(i)
        if i >= LAG:
            emit_back(i - LAG, states.pop(i - LAG))

    # ---- tail: Z, Zinv, h = R * Zinv
    zps = psp.tile([1, S], F32, tag="pr")
    for sj in range(NSJ):
        sl = slice(sj * SJ, (sj + 1) * SJ)
        nc.tensor.matmul(zps[:, sl], ones_bf[:], e_all[:, sl], start=True, stop=True)
    lnz = hp.tile([1, S], F32, tag="ho")
    nc.scalar.activation(lnz[:], zps[:], AF.Ln)
    ziv = hp.tile([1, S], F32, tag="ho")
    nc.scalar.activation(ziv[:], lnz[:], AF.Exp, scale=-1.0)
    zib = const.tile([P, S], F32, tag="zib")
    nc.gpsimd.partition_broadcast(zib[:], ziv[:], channels=P)

    ho = hp.tile([P, CC, S], F32, tag="ho")
    nc.vector.tensor_tensor(ho[:], r_tiles[N1 % 2][:],
                            zib[:].unsqueeze(1).to_broadcast((P, CC, S)), ALU.mult)
    nc.sync.dma_start(hout.rearrange("(a p) s -> p a s", a=2), ho[:])


_CACHE = {}


def _build(reps=1, no_cc=False):
    key = ("nc", reps, no_cc)
    if key in _CACHE:
        return _CACHE[key]
    nc = bacc.Bacc("TRN2", target_bir_lowering=False, debug=False, num_devices=8)
    v = nc.dram_tensor("v", [N1, C, S], F32, kind="ExternalInput").ap()
    w = nc.dram_tensor("w", [C], F32, kind="ExternalInput").ap()
    gnw = nc.dram_tensor("gnw", [C], F32, kind="ExternalInput").ap()
    gnb = nc.dram_tensor("gnb", [C], F32, kind="ExternalInput").ap()
    hout = nc.dram_tensor("h", [C, S], F32, kind="ExternalOutput").ap()
    with tile.TileContext(nc) as tc:
        for r in range(reps):
            _emit(tc, v, w, gnw, gnb, hout, rep=r, no_cc=no_cc)
    nc.compile()
    _CACHE[key] = nc
    return nc


def make_in_maps(blocks, x, w, gn_weight, gn_bias):
    blocks = np.asarray(blocks, dtype=np.float32)
    x = np.asarray(x, dtype=np.float32)
    w = np.asarray(w, dtype=np.float32)
    gn_weight = np.asarray(gn_weight, dtype=np.float32)
    gn_bias = np.asarray(gn_bias, dtype=np.float32)
    in_maps = []
    for k in range(8):
        b, hh = k // 2, k % 2
        hs = 32 * hh
        arr = np.empty((N1, C, S), np.float32)
        arr[:8] = blocks[:, b, :, hs:hs + 32, :].reshape(8, C, S)
        arr[8] = x[b, :, hs:hs + 32, :].reshape(C, S)
        in_maps.append({"v": arr, "w": w, "gnw": gn_weight, "gnb": gn_bias})
    return in_maps


def assemble(results):
    h = np.empty((4, C, 64, 64), np.float32)
    for k in range(8):
        b, hh = k // 2, k % 2
        h[b, :, 32 * hh:32 * hh + 32, :] = results[k]["h"].reshape(C, 32, 64)
    return h


def kernel(blocks, x, w, gn_weight, gn_bias):
    nc = _build()
    in_maps = make_in_maps(blocks, x, w, gn_weight, gn_bias)
    res = bass_utils.run_bass_kernel_spmd(nc, in_maps, core_ids=list(range(8)))
    return assemble(res.results)
